# revision 37
# baseline (speedup 1.0000x reference)
"""Trainium2 Bass kernel for a single-step GRU attention decoder.

Math (matches the reference nn.Module):
    xe  = emb[x]                                   # [H]
    a   = log_softmax(cat(xe, h0) @ attn_W.T + attn_b)   # [L]
    ap  = a @ encoder_outputs                      # [H]
    g   = relu(cat(xe, ap) @ ctx_W.T + ctx_b)      # [H]
    GRU(g, h0) -> h_new                            # [H]
    logits = h_new @ out_W.T + out_b               # [V]

Distribution across 8 NeuronCores (one TRN2 chip):
  - attention sharded over L (512 rows/core); exploiting linearity,
    log_softmax @ enc == a @ enc - (log sum exp a) * colsum(enc), so one
    AllReduce of per-core partials {a@enc, colsum(enc), sum(exp a)} lets
    every core reconstruct attn_applied locally.
  - ctx projection sharded by output rows; GRU mats sharded by *input*
    columns so a single AllReduce of partial (gi, gh) lets every core
    compute the full gates / h_new locally.
  - out projection sharded over vocab (6283 rows/core, padded to 6400).

Precision: weights are bf16 (TensorE runs bf16 stationary operands ~16x
faster than fp32, which decomposes into LOW/HIGH dual passes; bf16 also
halves HBM traffic). Because this module's log-softmax "attention"
produces huge activations (attn_applied ~1e3), the ctx->GRU chain uses
hi/lo residual bf16 (W ~ Whi + Wlo, x ~ xhi + xlo, keeping Whi*xhi +
Whi*xlo + Wlo*xhi) so gate pre-activations keep ~fp32 accuracy;
accumulation is always fp32 in PSUM, as are softmax/gates/biases.

All per-core slices are packed on the host so every device DMA is one
contiguous stream with long per-partition runs.
"""

import ml_dtypes
import numpy as np

import concourse.bass as bass
import concourse.bacc as bacc
import concourse.tile as tile
from concourse import mybir
from concourse.bass_utils import run_bass_kernel_spmd

H = 1024
V = 50257
L = 4096
NCORES = 8
LC = L // NCORES          # 512 encoder rows per core
HC = H // NCORES          # 128 hidden chunk per core
VC = -(-V // NCORES)      # 6283 vocab rows per core
VT = 50                   # vocab tiles of 128 per core
VPAD = VT * 128           # 6400
F32 = mybir.dt.float32
BF16 = mybir.dt.bfloat16
NPBF16 = ml_dtypes.bfloat16
RG = [list(range(NCORES))]

_CACHE = {}


def _build(dbg=False):
    key = ("nc", dbg)
    if key in _CACHE:
        return _CACHE[key]

    nc = bacc.Bacc("TRN2", target_bir_lowering=False, debug=False,
                   num_devices=NCORES)

    def din(name, shape, dt=F32):
        return nc.dram_tensor(name, shape, dt, kind="ExternalInput")

    catin_d = din("catin", [128, 16], BF16)    # cols 0-7 xe, 8-15 h0
    attn_wt_d = din("attn_wt", [128, 16 * LC], BF16)
    attn_b_d = din("attn_b", [128, 4])
    enc_hi_d = din("enc_hi", [128, 4 * H], BF16)
    enc_lo_d = din("enc_lo", [128, 4 * H], BF16)
    ctx_hi_d = din("ctx_hi", [128, 2048], BF16)
    ctx_lo_d = din("ctx_lo", [128, 1024], BF16)
    ctx_b_d = din("ctx_b", [128, 1])
    wih_hi_d = din("wih_hi", [HC, 3 * H], BF16)
    wih_lo_d = din("wih_lo", [HC, 3 * H], BF16)
    whh_d = din("whh", [HC, 3 * H], BF16)
    h0c_d = din("h0c", [128, 1], BF16)         # h0 chunk k (matmul operand)
    h0cm_d = din("h0cm", [128, 8])             # full h0, col-major, fp32
    bias8_d = din("bias8", [128, 48])          # cat(b_ih, b_hh)/8 col-major
    out_wt_d = din("out_wt", [H, VPAD], BF16)  # out_W vocab chunk, transposed
    out_b_d = din("out_b", [128, VT])
    out_d = nc.dram_tensor("out", [128, VT], F32, kind="ExternalOutput")

    AR = "AllReduce"
    ADD = mybir.AluOpType.add
    ACTF = mybir.ActivationFunctionType

    with tile.TileContext(nc) as tc:
        with (
            tc.tile_pool(name="wp", bufs=1) as wp,
            tc.tile_pool(name="sp", bufs=1) as sp,
            tc.tile_pool(name="pp", bufs=1, space="PSUM") as pp,
            tc.tile_pool(name="dp", bufs=1, space="DRAM") as dp,
        ):
            # ------------- loads (issue order = priority order) -------------
            catin = sp.tile([128, 16], BF16, tag="catin")
            nc.sync.dma_start(catin[:], catin_d[:])
            abias = sp.tile([128, 4], F32, tag="abias")
            nc.sync.dma_start(abias[:], attn_b_d[:])
            cbias = sp.tile([128, 1], F32, tag="cbias")
            nc.sync.dma_start(cbias[:], ctx_b_d[:])
            h0c = sp.tile([128, 1], BF16, tag="h0c")
            nc.sync.dma_start(h0c[:], h0c_d[:])
            h0cm = sp.tile([128, 8], F32, tag="h0cm")
            nc.sync.dma_start(h0cm[:], h0cm_d[:])
            bias8 = sp.tile([128, 48], F32, tag="bias8")
            nc.sync.dma_start(bias8[:], bias8_d[:])
            obias = sp.tile([128, VT], F32, tag="obias")
            nc.sync.dma_start(obias[:], out_b_d[:])

            attn_sb = wp.tile([128, 16 * LC], BF16, tag="attn")
            nc.sync.dma_start(attn_sb[:, :8 * LC], attn_wt_d[:, :8 * LC])
            nc.sync.dma_start(attn_sb[:, 8 * LC:], attn_wt_d[:, 8 * LC:])
            enc_hi = wp.tile([128, 4 * H], BF16, tag="enchi")
            nc.sync.dma_start(enc_hi[:], enc_hi_d[:])
            enc_lo = wp.tile([128, 4 * H], BF16, tag="enclo")
            nc.sync.dma_start(enc_lo[:], enc_lo_d[:])
            ctx_hi = wp.tile([128, 2048], BF16, tag="ctxhi")
            nc.sync.dma_start(ctx_hi[:], ctx_hi_d[:])
            ctx_lo = wp.tile([128, 1024], BF16, tag="ctxlo")
            nc.sync.dma_start(ctx_lo[:], ctx_lo_d[:])
            wih_hi = wp.tile([128, 3 * H], BF16, tag="wihhi")
            nc.sync.dma_start(wih_hi[:], wih_hi_d[:])
            wih_lo = wp.tile([128, 3 * H], BF16, tag="wihlo")
            nc.sync.dma_start(wih_lo[:], wih_lo_d[:])
            whh_sb = wp.tile([128, 3 * H], BF16, tag="whh")
            nc.sync.dma_start(whh_sb[:], whh_d[:])

            # out_W: 8 contiguous whole-h-tile streams, SBUF resident
            outw = []
            for j in range(8):
                t = wp.tile([128, VPAD], BF16, tag=f"outw{j}")
                nc.sync.dma_start(t[:], out_wt_d[128 * j:128 * (j + 1), :])
                outw.append(t)

            # ---------------- attention logits ----------------
            # a[l] for the 512 local l, laid out [128, 4] col-major tiles.
            # NOTE: accumulation groups must be contiguous in program order —
            # interleaving groups within one PSUM bank gives wrong results.
            # Split the contraction in two halves (xe tiles / h0 tiles) so the
            # PE can start as soon as the first half of attn_wt lands.
            a_psA = pp.tile([128, 4], F32, tag="apsA")
            for j in range(4):           # l tiles
                for i in range(8):       # xe half of the cat dim
                    nc.tensor.matmul(
                        a_psA[:, j:j + 1],
                        attn_sb[:, LC * i + 128 * j:LC * i + 128 * (j + 1)],
                        catin[:, i:i + 1],
                        start=(i == 0), stop=(i == 7))
            a_psB = pp.tile([128, 4], F32, tag="apsB")
            for j in range(4):           # l tiles
                for i in range(8, 16):   # h0 half
                    nc.tensor.matmul(
                        a_psB[:, j:j + 1],
                        attn_sb[:, LC * i + 128 * j:LC * i + 128 * (j + 1)],
                        catin[:, i:i + 1],
                        start=(i == 8), stop=(i == 15))
            a_half = sp.tile([128, 4], F32, tag="ahalf")
            nc.vector.tensor_add(a_half[:], a_psA[:], abias[:])
            a_sb = sp.tile([128, 4], F32, tag="a")
            nc.vector.tensor_add(a_sb[:], a_psB[:], a_half[:])
            exp_sb = sp.tile([128, 4], F32, tag="expa")
            nc.scalar.activation(exp_sb[:], a_sb[:], ACTF.Exp)

            # split a into hi/lo bf16
            a_hi = sp.tile([128, 4], BF16, tag="ahi")
            nc.vector.tensor_copy(a_hi[:], a_sb[:])
            a_hif = sp.tile([128, 4], F32, tag="ahif")
            nc.vector.tensor_copy(a_hif[:], a_hi[:])
            a_lof = sp.tile([128, 4], F32, tag="alof")
            nc.vector.tensor_sub(a_lof[:], a_sb[:], a_hif[:])
            # rhs3: col 3j = a_hi_j, 3j+1 = a_lo_j, 3j+2 = ones
            rhs3 = sp.tile([128, 12], BF16, tag="rhs3")
            nc.vector.tensor_copy(rhs3[:, 0:12:3], a_hi[:])
            nc.vector.tensor_copy(rhs3[:, 1:12:3], a_lof[:])
            nc.vector.memset(rhs3[:, 2:12:3], 1.0)

            # pack psum cols per h-chunk c: 5c+0 ehi*ahi, +1 ehi*alo,
            # +2 ehi*1, +3 elo*ahi, +4 elo*1
            pack_ps = pp.tile([128, 40], F32, tag="packps")
            for c in range(8):
                for j in range(4):
                    nc.tensor.matmul(
                        pack_ps[:, 5 * c:5 * c + 3],
                        enc_hi[:, H * j + 128 * c:H * j + 128 * (c + 1)],
                        rhs3[:, 3 * j:3 * j + 3],
                        start=(j == 0), stop=(j == 3))
                for j in range(4):
                    nc.tensor.matmul(
                        pack_ps[:, 5 * c + 3:5 * c + 5],
                        enc_lo[:, H * j + 128 * c:H * j + 128 * (c + 1)],
                        rhs3[:, 3 * j:3 * j + 3:2],
                        start=(j == 0), stop=(j == 3))
            # sum over partitions+cols of exp(a) -> s
            exp_r = sp.tile([128, 1], F32, tag="expr")
            nc.vector.reduce_sum(exp_r[:], exp_sb[:], axis=mybir.AxisListType.X)
            s_sb = sp.tile([1, 1], F32, tag="s")
            nc.gpsimd.tensor_reduce(s_sb[:], exp_r[:],
                                    axis=mybir.AxisListType.C,
                                    op=mybir.AluOpType.add)

            pk_sb = sp.tile([128, 40], F32, tag="pksb")
            nc.vector.tensor_copy(pk_sb[:], pack_ps[:])
            pack_sb = sp.tile([128, 18], F32, tag="pack")
            nc.vector.tensor_add(pack_sb[:, 0:8], pk_sb[:, 0:40:5],
                                 pk_sb[:, 1:40:5])
            nc.vector.tensor_add(pack_sb[:, 0:8], pack_sb[:, 0:8],
                                 pk_sb[:, 3:40:5])
            nc.vector.tensor_add(pack_sb[:, 8:16], pk_sb[:, 2:40:5],
                                 pk_sb[:, 4:40:5])
            nc.vector.memset(pack_sb[:, 16:18], 0.0)
            nc.vector.tensor_copy(pack_sb[0:1, 16:17], s_sb[:])

            # gh = W_hh @ h0 depends only on inputs — run it here so the PE
            # does it inside the CC1 wait instead of on the critical path.
            gih_ps = pp.tile([128, 48], F32, tag="gihps")
            for c in range(24):
                nc.tensor.matmul(gih_ps[:, 24 + c:25 + c],
                                 whh_sb[:, 128 * c:128 * (c + 1)], h0c[:],
                                 start=True, stop=True)

            # ---------------- collective 1: AllReduce(add) ----------------
            cc1_in = dp.tile([128, 18], F32, tag="cc1in")
            cc1_out = dp.tile([128, 18], F32, tag="cc1out",
                              addr_space="Shared")
            nc.gpsimd.dma_start(cc1_in[:], pack_sb[:])
            nc.gpsimd.collective_compute(AR, ADD, replica_groups=RG,
                                         ins=[cc1_in.opt()],
                                         outs=[cc1_out.opt()])
            sums = sp.tile([128, 18], F32, tag="sums")
            nc.gpsimd.dma_start(sums[:], cc1_out[:])

            # c = log(sum exp), broadcast to all partitions via PE
            c_sb = sp.tile([1, 1], F32, tag="c")
            nc.scalar.activation(c_sb[:], sums[0:1, 16:17], ACTF.Ln)
            ones_r = sp.tile([1, 128], F32, tag="ones_r")
            nc.vector.memset(ones_r[:], 1.0)
            cb_ps = pp.tile([128, 1], F32, tag="cbps")
            nc.tensor.matmul(cb_ps[:], ones_r[:], c_sb[:])
            cb_sb = sp.tile([128, 1], F32, tag="cb")
            nc.vector.tensor_copy(cb_sb[:], cb_ps[:])

            # attn_applied = partial1_sum - c * colsum_sum   [128, 8]
            atmp = sp.tile([128, 8], F32, tag="atmp")
            nc.vector.tensor_scalar(atmp[:], sums[:, 8:16], cb_sb[:], None,
                                    mybir.AluOpType.mult)
            attnap = sp.tile([128, 8], F32, tag="attnap")
            nc.vector.tensor_sub(attnap[:], sums[:, 0:8], atmp[:])
            # split attnap hi/lo bf16
            ap_hi = sp.tile([128, 8], BF16, tag="aphi")
            nc.vector.tensor_copy(ap_hi[:], attnap[:])
            ap_hif = sp.tile([128, 8], F32, tag="aphif")
            nc.vector.tensor_copy(ap_hif[:], ap_hi[:])
            ap_lof = sp.tile([128, 8], F32, tag="aplof")
            nc.vector.tensor_sub(ap_lof[:], attnap[:], ap_hif[:])
            ap_lo = sp.tile([128, 8], BF16, tag="aplo")
            nc.vector.tensor_copy(ap_lo[:], ap_lof[:])

            # ---------------- context projection ----------------
            # one accumulation group: xe + attnap hi/lo residual products
            g_ps = pp.tile([128, 1], F32, tag="gps")
            nmm = 8 + 8 + 8 + 8
            k = 0
            for i in range(8):
                nc.tensor.matmul(g_ps[:], ctx_hi[:, 128 * i:128 * (i + 1)],
                                 catin[:, i:i + 1],
                                 start=(k == 0), stop=(k == nmm - 1))
                k += 1
            for m in range(8):
                t = ctx_hi[:, 128 * (8 + m):128 * (9 + m)]
                nc.tensor.matmul(g_ps[:], t, ap_hi[:, m:m + 1],
                                 start=False, stop=(k == nmm - 1))
                k += 1
                nc.tensor.matmul(g_ps[:], t, ap_lo[:, m:m + 1],
                                 start=False, stop=(k == nmm - 1))
                k += 1
            for m in range(8):
                nc.tensor.matmul(g_ps[:], ctx_lo[:, 128 * m:128 * (m + 1)],
                                 ap_hi[:, m:m + 1],
                                 start=False, stop=(k == nmm - 1))
                k += 1
            g_sb = sp.tile([128, 1], F32, tag="g")
            nc.scalar.activation(g_sb[:], g_ps[:], ACTF.Relu, bias=cbias[:])
            # split g hi/lo bf16
            g_hi = sp.tile([128, 1], BF16, tag="ghi")
            nc.vector.tensor_copy(g_hi[:], g_sb[:])
            g_hif = sp.tile([128, 1], F32, tag="ghif")
            nc.vector.tensor_copy(g_hif[:], g_hi[:])
            g_lof = sp.tile([128, 1], F32, tag="glof")
            nc.vector.tensor_sub(g_lof[:], g_sb[:], g_hif[:])
            g_lo = sp.tile([128, 1], BF16, tag="glo")
            nc.vector.tensor_copy(g_lo[:], g_lof[:])

            # ---------------- GRU partial matvecs (gi; gh ran earlier) ----
            for c in range(24):
                whi = wih_hi[:, 128 * c:128 * (c + 1)]
                nc.tensor.matmul(gih_ps[:, c:c + 1], whi, g_hi[:],
                                 start=True, stop=False)
                nc.tensor.matmul(gih_ps[:, c:c + 1], whi, g_lo[:],
                                 start=False, stop=False)
                nc.tensor.matmul(gih_ps[:, c:c + 1],
                                 wih_lo[:, 128 * c:128 * (c + 1)], g_hi[:],
                                 start=False, stop=True)
            # fold biases/8 in here: the 8-way AllReduce sum restores them
            pack2 = sp.tile([128, 48], F32, tag="pack2")
            nc.vector.tensor_add(pack2[:], gih_ps[:], bias8[:])

            # ---------------- collective 2: AllReduce(add) ----------------
            cc2_in = dp.tile([128, 48], F32, tag="cc2in")
            cc2_out = dp.tile([128, 48], F32, tag="cc2out",
                              addr_space="Shared")
            nc.gpsimd.dma_start(cc2_in[:], pack2[:])
            nc.gpsimd.collective_compute(AR, ADD, replica_groups=RG,
                                         ins=[cc2_in.opt()],
                                         outs=[cc2_out.opt()])
            gsum = sp.tile([128, 48], F32, tag="gsum")
            nc.gpsimd.dma_start(gsum[:], cc2_out[:])

            # Keep the PE HAM-warm through the CC2 wait so the out-projection
            # runs at 2.4 GHz: ~13us of filler matmuls gated only on pack2.
            # The consumer that keeps this from being dead-code-eliminated is
            # emitted at the very end so it doesn't block any engine FIFO.
            gz = sp.tile([128, 1], BF16, tag="gz")
            nc.vector.tensor_copy(gz[:], pack2[:, 0:1])
            warm_ps = pp.tile([128, 1], F32, tag="apsA")  # attention bank,
            NWARM = 480                                   # free by now
            for w in range(NWARM):
                nc.tensor.matmul(warm_ps[:],
                                 outw[0][:, 128 * (w % 50):128 * (w % 50 + 1)],
                                 gz[:], start=(w == 0), stop=(w == NWARM - 1))

            # gates (PyTorch order r, z, n); r and z share one Sigmoid pass
            rzpre = sp.tile([128, 16], F32, tag="rzpre")
            nc.vector.tensor_add(rzpre[:], gsum[:, 0:16], gsum[:, 24:40])
            rz_sb = sp.tile([128, 16], F32, tag="rz")
            nc.scalar.activation(rz_sb[:], rzpre[:], ACTF.Sigmoid)
            npre = sp.tile([128, 8], F32, tag="npre")
            nc.vector.tensor_mul(npre[:], rz_sb[:, 0:8], gsum[:, 40:48])
            nc.vector.tensor_add(npre[:], npre[:], gsum[:, 16:24])
            n_sb = sp.tile([128, 8], F32, tag="n")
            nc.scalar.activation(n_sb[:], npre[:], ACTF.Tanh)
            # h_new = n + z * (h0 - n)
            hd = sp.tile([128, 8], F32, tag="hd")
            nc.vector.tensor_sub(hd[:], h0cm[:], n_sb[:])
            nc.vector.tensor_mul(hd[:], hd[:], rz_sb[:, 8:16])
            hnew_b = sp.tile([128, 8], BF16, tag="hnewb")
            nc.vector.tensor_add(hnew_b[:], n_sb[:], hd[:])

            # ---------------- output projection ----------------
            logit_sb = sp.tile([128, VT], F32, tag="logit")
            t0c = 0
            while t0c < VT:
                nt = min(8, VT - t0c)
                o_ps = pp.tile([128, nt], F32, tag="ops", bufs=2)
                for t in range(t0c, t0c + nt):
                    for j in range(8):
                        nc.tensor.matmul(
                            o_ps[:, t - t0c:t - t0c + 1],
                            outw[j][:, 128 * t:128 * (t + 1)],
                            hnew_b[:, j:j + 1],
                            start=(j == 0), stop=(j == 7))
                nc.vector.tensor_add(logit_sb[:, t0c:t0c + nt], o_ps[:],
                                     obias[:, t0c:t0c + nt])
                t0c += nt

            nc.gpsimd.dma_start(out_d[:], logit_sb[:])

            # warm-block consumer, placed last so its engine-FIFO slots
            # come after all real work
            warm_out = sp.tile([128, 1], F32, tag="warmout")
            nc.vector.tensor_copy(warm_out[:], warm_ps[:])
            warm_dump = dp.tile([128, 1], F32, tag="warmdump")
            nc.gpsimd.dma_start(warm_dump[:], warm_out[:])

            if dbg:
                dbg_tiles = {
                    "dbg_a": a_sb, "dbg_exp": exp_sb, "dbg_pack": pack_sb,
                    "dbg_sums": sums, "dbg_cb": cb_sb, "dbg_attnap": attnap,
                    "dbg_g": g_sb, "dbg_pack2": pack2, "dbg_gsum": gsum,
                    "dbg_hnew": hnew_b,
                }
                for name, t in dbg_tiles.items():
                    shp = list(t[:].shape)
                    d = nc.dram_tensor(name, shp, t[:].dtype,
                                       kind="ExternalOutput")
                    nc.sync.dma_start(d[:], t[:])

    nc.compile()
    _CACHE[key] = nc
    return nc


def _col_major(v, ncols):
    # v [n] -> [128, ncols] with [p, c] = v[128 * c + p]
    return np.ascontiguousarray(v.reshape(ncols, 128).T)


def _pack_rows(a, nb):
    # a [nb*128, w] -> [128, nb*w] with [p, w*i + q] = a[128*i + p, q]
    w = a.shape[1]
    return np.ascontiguousarray(
        a.reshape(nb, 128, w).transpose(1, 0, 2).reshape(128, nb * w))


def _hi_lo(a):
    hi = a.astype(NPBF16)
    lo = (a - hi.astype(np.float32)).astype(NPBF16)
    return hi, lo


def _shard(inputs):
    x = np.asarray(inputs["x"]).reshape(-1)
    h0 = np.asarray(inputs["h"], dtype=np.float32).reshape(H)
    enc = np.asarray(inputs["encoder_outputs"], dtype=np.float32)
    emb = np.asarray(inputs["emb"])
    attn_W = np.asarray(inputs["attn_W"], dtype=np.float32)
    attn_b = np.asarray(inputs["attn_b"], dtype=np.float32)
    ctx_W = np.asarray(inputs["ctx_W"], dtype=np.float32)
    ctx_b = np.asarray(inputs["ctx_b"], dtype=np.float32)
    W_ih = np.asarray(inputs["W_ih"], dtype=np.float32)
    W_hh = np.asarray(inputs["W_hh"], dtype=np.float32)
    b_ih = np.asarray(inputs["b_ih"], dtype=np.float32)
    b_hh = np.asarray(inputs["b_hh"], dtype=np.float32)
    out_W = np.asarray(inputs["out_W"], dtype=np.float32)
    out_b = np.asarray(inputs["out_b"], dtype=np.float32)

    xe = np.asarray(emb[int(x[0])], dtype=np.float32)
    catin = np.concatenate([_col_major(xe, 8), _col_major(h0, 8)],
                           axis=1).astype(NPBF16)
    h0cm = _col_major(h0, 8)

    bias8 = np.concatenate([_col_major(b_ih, 24), _col_major(b_hh, 24)],
                           axis=1) / 8.0

    in_maps = []
    for k in range(NCORES):
        lsl = slice(LC * k, LC * (k + 1))
        hsl = slice(HC * k, HC * (k + 1))
        v0, v1 = VC * k, min(VC * (k + 1), V)
        owt = np.zeros((H, VPAD), dtype=NPBF16)
        owt[:, :v1 - v0] = out_W[v0:v1, :].T.astype(NPBF16)
        ob = np.zeros(VPAD, dtype=np.float32)
        ob[:v1 - v0] = out_b[v0:v1]

        enc_hi, enc_lo = _hi_lo(enc[lsl, :])
        ctxT = np.ascontiguousarray(ctx_W[hsl, :].T)       # [2048, 128]
        ctx_hi = ctxT.astype(NPBF16)
        ctx_lo = (ctxT[H:] - ctx_hi[H:].astype(np.float32)).astype(NPBF16)
        wihT = np.ascontiguousarray(W_ih[:, hsl].T)        # [128, 3072]
        wih_hi, wih_lo = _hi_lo(wihT)

        in_maps.append({
            "catin": np.ascontiguousarray(catin),
            "attn_wt": _pack_rows(attn_W[lsl, :].T.astype(NPBF16), 16),
            "attn_b": _col_major(attn_b[lsl], 4),
            "enc_hi": _pack_rows(enc_hi, 4),
            "enc_lo": _pack_rows(enc_lo, 4),
            "ctx_hi": _pack_rows(ctx_hi, 16),
            "ctx_lo": _pack_rows(ctx_lo, 8),
            "ctx_b": ctx_b[hsl].reshape(128, 1).copy(),
            "wih_hi": wih_hi,
            "wih_lo": wih_lo,
            "whh": np.ascontiguousarray(W_hh[:, hsl].T).astype(NPBF16),
            "h0c": h0[hsl].reshape(128, 1).astype(NPBF16),
            "h0cm": h0cm,
            "bias8": np.ascontiguousarray(bias8, dtype=np.float32),
            "out_wt": owt,
            "out_b": _col_major(ob, VT),
        })
    return in_maps


def _gather(results):
    logits = np.empty(NCORES * VC, dtype=np.float32)
    for k in range(NCORES):
        chunk = np.asarray(results[k]["out"]).T.ravel()   # [VT*128]
        logits[VC * k:VC * (k + 1)] = chunk[:VC]
    return logits[:V].reshape(1, V)


def kernel(**inputs):
    nc = _build()
    in_maps = _shard(inputs)
    try:
        res = run_bass_kernel_spmd(nc, in_maps, core_ids=list(range(NCORES)))
    except Exception:
        # A dirty device state from a previous process occasionally fails
        # the first launch (NRT_EXEC_UNIT_UNRECOVERABLE); one retry clears.
        res = run_bass_kernel_spmd(nc, in_maps, core_ids=list(range(NCORES)))
    return _gather(res.results)


def kernel_traced(**inputs):
    """Like kernel() but profiles on HW; returns (output, exec_time_ns)."""
    nc = _build()
    in_maps = _shard(inputs)
    res = run_bass_kernel_spmd(nc, in_maps, core_ids=list(range(NCORES)),
                               trace=True)
    return _gather(res.results), res.exec_time_ns


def kernel_debug(**inputs):
    """Run the debug build; returns per-core dicts of all outputs."""
    nc = _build(dbg=True)
    in_maps = _shard(inputs)
    res = run_bass_kernel_spmd(nc, in_maps, core_ids=list(range(NCORES)))
    return res.results


# revision 39
# speedup vs baseline: 1.1413x; 1.1413x over previous
"""Trainium2 Bass kernel for a single-step GRU attention decoder.

Math (matches the reference nn.Module):
    xe  = emb[x]                                   # [H]
    a   = log_softmax(cat(xe, h0) @ attn_W.T + attn_b)   # [L]
    ap  = a @ encoder_outputs                      # [H]
    g   = relu(cat(xe, ap) @ ctx_W.T + ctx_b)      # [H]
    GRU(g, h0) -> h_new                            # [H]
    logits = h_new @ out_W.T + out_b               # [V]

Distribution across 8 NeuronCores (one TRN2 chip):
  - attention sharded over L (512 rows/core); exploiting linearity,
    log_softmax @ enc == a @ enc - (log sum exp a) * colsum(enc), so one
    AllReduce of per-core partials {a@enc, colsum(enc), sum(exp a)} lets
    every core reconstruct attn_applied locally.
  - ctx projection sharded by output rows; GRU mats sharded by *input*
    columns so a single AllReduce of partial (gi, gh) lets every core
    compute the full gates / h_new locally.
  - out projection sharded over vocab (6283 rows/core, padded to 6400).

Precision: weights are bf16 (TensorE runs bf16 stationary operands ~16x
faster than fp32, which decomposes into LOW/HIGH dual passes; bf16 also
halves HBM traffic). Because this module's log-softmax "attention"
produces huge activations (attn_applied ~1e3), the ctx->GRU chain uses
hi/lo residual bf16 (W ~ Whi + Wlo, x ~ xhi + xlo, keeping Whi*xhi +
Whi*xlo + Wlo*xhi) so gate pre-activations keep ~fp32 accuracy;
accumulation is always fp32 in PSUM, as are softmax/gates/biases.

All per-core slices are packed on the host so every device DMA is one
contiguous stream with long per-partition runs.
"""

import ml_dtypes
import numpy as np

import concourse.bass as bass
import concourse.bacc as bacc
import concourse.tile as tile
from concourse import mybir
from concourse.bass_utils import run_bass_kernel_spmd

H = 1024
V = 50257
L = 4096
NCORES = 8
LC = L // NCORES          # 512 encoder rows per core
HC = H // NCORES          # 128 hidden chunk per core
VC = -(-V // NCORES)      # 6283 vocab rows per core
VT = 50                   # vocab tiles of 128 per core
VPAD = VT * 128           # 6400
F32 = mybir.dt.float32
BF16 = mybir.dt.bfloat16
NPBF16 = ml_dtypes.bfloat16
RG = [list(range(NCORES))]

_CACHE = {}


def _build(dbg=False):
    key = ("nc", dbg)
    if key in _CACHE:
        return _CACHE[key]

    nc = bacc.Bacc("TRN2", target_bir_lowering=False, debug=False,
                   num_devices=NCORES)

    def din(name, shape, dt=F32):
        return nc.dram_tensor(name, shape, dt, kind="ExternalInput")

    catin_d = din("catin", [128, 16], BF16)    # cols 0-7 xe, 8-15 h0
    attn_wt_d = din("attn_wt", [128, 16 * LC], BF16)
    attn_b_d = din("attn_b", [128, 4])
    enc_hi_d = din("enc_hi", [128, 4 * H], BF16)
    enc_lo_d = din("enc_lo", [128, 4 * H], BF16)
    ctx_hi_d = din("ctx_hi", [128, 2048], BF16)
    ctx_lo_d = din("ctx_lo", [128, 1024], BF16)
    ctx_b_d = din("ctx_b", [128, 1])
    wih_hi_d = din("wih_hi", [HC, 3 * H], BF16)
    wih_lo_d = din("wih_lo", [HC, 3 * H], BF16)
    whh_d = din("whh", [HC, 3 * H], BF16)
    h0c_d = din("h0c", [128, 1], BF16)         # h0 chunk k (matmul operand)
    h0cm_d = din("h0cm", [128, 8])             # full h0, col-major, fp32
    bias8_d = din("bias8", [128, 48])          # cat(b_ih, b_hh)/8 col-major
    out_wt_d = din("out_wt", [H, VPAD], BF16)  # out_W vocab chunk, transposed
    out_b_d = din("out_b", [128, VT])
    out_d = nc.dram_tensor("out", [128, VT], F32, kind="ExternalOutput")

    AR = "AllReduce"
    ADD = mybir.AluOpType.add
    ACTF = mybir.ActivationFunctionType

    with tile.TileContext(nc) as tc:
        with (
            tc.tile_pool(name="wp", bufs=1) as wp,
            tc.tile_pool(name="sp", bufs=1) as sp,
            tc.tile_pool(name="pp", bufs=1, space="PSUM") as pp,
            tc.tile_pool(name="dp", bufs=1, space="DRAM") as dp,
        ):
            # ------------- loads (issue order = priority order) -------------
            catin = sp.tile([128, 16], BF16, tag="catin")
            nc.sync.dma_start(catin[:], catin_d[:])
            abias = sp.tile([128, 4], F32, tag="abias")
            nc.sync.dma_start(abias[:], attn_b_d[:])
            cbias = sp.tile([128, 1], F32, tag="cbias")
            nc.sync.dma_start(cbias[:], ctx_b_d[:])
            h0c = sp.tile([128, 1], BF16, tag="h0c")
            nc.sync.dma_start(h0c[:], h0c_d[:])
            h0cm = sp.tile([128, 8], F32, tag="h0cm")
            nc.sync.dma_start(h0cm[:], h0cm_d[:])
            bias8 = sp.tile([128, 48], F32, tag="bias8")
            nc.sync.dma_start(bias8[:], bias8_d[:])
            obias = sp.tile([128, VT], F32, tag="obias")
            nc.sync.dma_start(obias[:], out_b_d[:])

            attn_sb = wp.tile([128, 16 * LC], BF16, tag="attn")
            nc.sync.dma_start(attn_sb[:, :8 * LC], attn_wt_d[:, :8 * LC])
            nc.sync.dma_start(attn_sb[:, 8 * LC:], attn_wt_d[:, 8 * LC:])
            enc_hi = wp.tile([128, 4 * H], BF16, tag="enchi")
            nc.sync.dma_start(enc_hi[:], enc_hi_d[:])
            enc_lo = wp.tile([128, 4 * H], BF16, tag="enclo")
            nc.sync.dma_start(enc_lo[:], enc_lo_d[:])
            ctx_hi = wp.tile([128, 2048], BF16, tag="ctxhi")
            nc.sync.dma_start(ctx_hi[:], ctx_hi_d[:])
            ctx_lo = wp.tile([128, 1024], BF16, tag="ctxlo")
            nc.sync.dma_start(ctx_lo[:], ctx_lo_d[:])
            wih_hi = wp.tile([128, 3 * H], BF16, tag="wihhi")
            nc.sync.dma_start(wih_hi[:], wih_hi_d[:])
            wih_lo = wp.tile([128, 3 * H], BF16, tag="wihlo")
            nc.sync.dma_start(wih_lo[:], wih_lo_d[:])
            whh_sb = wp.tile([128, 3 * H], BF16, tag="whh")
            nc.sync.dma_start(whh_sb[:], whh_d[:])

            # out_W: 8 contiguous whole-h-tile streams, SBUF resident
            outw = []
            for j in range(8):
                t = wp.tile([128, VPAD], BF16, tag=f"outw{j}")
                nc.sync.dma_start(t[:], out_wt_d[128 * j:128 * (j + 1), :])
                outw.append(t)

            # ---------------- attention logits ----------------
            # a[l] for the 512 local l, laid out [128, 4] col-major tiles.
            # NOTE: accumulation groups must be contiguous in program order —
            # interleaving groups within one PSUM bank gives wrong results.
            # Split the contraction in two halves (xe tiles / h0 tiles) so the
            # PE can start as soon as the first half of attn_wt lands.
            a_psA = pp.tile([128, 4], F32, tag="apsA")
            for j in range(4):           # l tiles
                for i in range(8):       # xe half of the cat dim
                    nc.tensor.matmul(
                        a_psA[:, j:j + 1],
                        attn_sb[:, LC * i + 128 * j:LC * i + 128 * (j + 1)],
                        catin[:, i:i + 1],
                        start=(i == 0), stop=(i == 7))
            a_psB = pp.tile([128, 4], F32, tag="apsB")
            for j in range(4):           # l tiles
                for i in range(8, 16):   # h0 half
                    nc.tensor.matmul(
                        a_psB[:, j:j + 1],
                        attn_sb[:, LC * i + 128 * j:LC * i + 128 * (j + 1)],
                        catin[:, i:i + 1],
                        start=(i == 8), stop=(i == 15))
            a_half = sp.tile([128, 4], F32, tag="ahalf")
            nc.vector.tensor_add(a_half[:], a_psA[:], abias[:])
            a_sb = sp.tile([128, 4], F32, tag="a")
            nc.vector.tensor_add(a_sb[:], a_psB[:], a_half[:])
            exp_sb = sp.tile([128, 4], F32, tag="expa")
            nc.scalar.activation(exp_sb[:], a_sb[:], ACTF.Exp)

            # split a into hi/lo bf16
            a_hi = sp.tile([128, 4], BF16, tag="ahi")
            nc.vector.tensor_copy(a_hi[:], a_sb[:])
            a_hif = sp.tile([128, 4], F32, tag="ahif")
            nc.vector.tensor_copy(a_hif[:], a_hi[:])
            a_lof = sp.tile([128, 4], F32, tag="alof")
            nc.vector.tensor_sub(a_lof[:], a_sb[:], a_hif[:])
            # rhs3: col 3j = a_hi_j, 3j+1 = a_lo_j, 3j+2 = ones
            rhs3 = sp.tile([128, 12], BF16, tag="rhs3")
            nc.vector.tensor_copy(rhs3[:, 0:12:3], a_hi[:])
            nc.vector.tensor_copy(rhs3[:, 1:12:3], a_lof[:])
            nc.vector.memset(rhs3[:, 2:12:3], 1.0)

            # pack psum cols per h-chunk c: 5c+0 ehi*ahi, +1 ehi*alo,
            # +2 ehi*1, +3 elo*ahi, +4 elo*1
            pack_ps = pp.tile([128, 40], F32, tag="packps")
            for c in range(8):
                for j in range(4):
                    nc.tensor.matmul(
                        pack_ps[:, 5 * c:5 * c + 3],
                        enc_hi[:, H * j + 128 * c:H * j + 128 * (c + 1)],
                        rhs3[:, 3 * j:3 * j + 3],
                        start=(j == 0), stop=(j == 3))
                for j in range(4):
                    nc.tensor.matmul(
                        pack_ps[:, 5 * c + 3:5 * c + 5],
                        enc_lo[:, H * j + 128 * c:H * j + 128 * (c + 1)],
                        rhs3[:, 3 * j:3 * j + 3:2],
                        start=(j == 0), stop=(j == 3))
            # sum over partitions+cols of exp(a) -> s
            exp_r = sp.tile([128, 1], F32, tag="expr")
            nc.vector.reduce_sum(exp_r[:], exp_sb[:], axis=mybir.AxisListType.X)
            s_sb = sp.tile([1, 1], F32, tag="s")
            nc.gpsimd.tensor_reduce(s_sb[:], exp_r[:],
                                    axis=mybir.AxisListType.C,
                                    op=mybir.AluOpType.add)

            pk_sb = sp.tile([128, 40], F32, tag="pksb")
            nc.vector.tensor_copy(pk_sb[:], pack_ps[:])
            pack_sb = sp.tile([128, 18], F32, tag="pack")
            nc.vector.tensor_add(pack_sb[:, 0:8], pk_sb[:, 0:40:5],
                                 pk_sb[:, 1:40:5])
            nc.vector.tensor_add(pack_sb[:, 0:8], pack_sb[:, 0:8],
                                 pk_sb[:, 3:40:5])
            nc.vector.tensor_add(pack_sb[:, 8:16], pk_sb[:, 2:40:5],
                                 pk_sb[:, 4:40:5])
            nc.vector.memset(pack_sb[:, 16:18], 0.0)
            nc.vector.tensor_copy(pack_sb[0:1, 16:17], s_sb[:])

            # gh = W_hh @ h0 depends only on inputs — run it here so the PE
            # does it inside the CC1 wait instead of on the critical path.
            gih_ps = pp.tile([128, 48], F32, tag="gihps")
            for c in range(24):
                nc.tensor.matmul(gih_ps[:, 24 + c:25 + c],
                                 whh_sb[:, 128 * c:128 * (c + 1)], h0c[:],
                                 start=True, stop=True)

            # ---------------- collective 1: AllReduce(add) ----------------
            cc1_in = dp.tile([128, 18], F32, tag="cc1in")
            cc1_out = dp.tile([128, 18], F32, tag="cc1out",
                              addr_space="Shared")
            nc.gpsimd.dma_start(cc1_in[:], pack_sb[:])
            nc.gpsimd.collective_compute(AR, ADD, replica_groups=RG,
                                         ins=[cc1_in.opt()],
                                         outs=[cc1_out.opt()])
            sums = sp.tile([128, 18], F32, tag="sums")
            nc.gpsimd.dma_start(sums[:], cc1_out[:])

            # c = log(sum exp), broadcast to all partitions via PE
            c_sb = sp.tile([1, 1], F32, tag="c")
            nc.scalar.activation(c_sb[:], sums[0:1, 16:17], ACTF.Ln)
            ones_r = sp.tile([1, 128], F32, tag="ones_r")
            nc.vector.memset(ones_r[:], 1.0)
            cb_ps = pp.tile([128, 1], F32, tag="cbps")
            nc.tensor.matmul(cb_ps[:], ones_r[:], c_sb[:])
            cb_sb = sp.tile([128, 1], F32, tag="cb")
            nc.vector.tensor_copy(cb_sb[:], cb_ps[:])

            # attn_applied = partial1_sum - c * colsum_sum   [128, 8]
            atmp = sp.tile([128, 8], F32, tag="atmp")
            nc.vector.tensor_scalar(atmp[:], sums[:, 8:16], cb_sb[:], None,
                                    mybir.AluOpType.mult)
            attnap = sp.tile([128, 8], F32, tag="attnap")
            nc.vector.tensor_sub(attnap[:], sums[:, 0:8], atmp[:])
            # split attnap hi/lo bf16
            ap_hi = sp.tile([128, 8], BF16, tag="aphi")
            nc.vector.tensor_copy(ap_hi[:], attnap[:])
            ap_hif = sp.tile([128, 8], F32, tag="aphif")
            nc.vector.tensor_copy(ap_hif[:], ap_hi[:])
            ap_lof = sp.tile([128, 8], F32, tag="aplof")
            nc.vector.tensor_sub(ap_lof[:], attnap[:], ap_hif[:])
            ap_lo = sp.tile([128, 8], BF16, tag="aplo")
            nc.vector.tensor_copy(ap_lo[:], ap_lof[:])

            # ---------------- context projection ----------------
            # one accumulation group: xe + attnap hi/lo residual products
            g_ps = pp.tile([128, 1], F32, tag="gps")
            nmm = 8 + 8 + 8 + 8
            k = 0
            for i in range(8):
                nc.tensor.matmul(g_ps[:], ctx_hi[:, 128 * i:128 * (i + 1)],
                                 catin[:, i:i + 1],
                                 start=(k == 0), stop=(k == nmm - 1))
                k += 1
            for m in range(8):
                t = ctx_hi[:, 128 * (8 + m):128 * (9 + m)]
                nc.tensor.matmul(g_ps[:], t, ap_hi[:, m:m + 1],
                                 start=False, stop=(k == nmm - 1))
                k += 1
                nc.tensor.matmul(g_ps[:], t, ap_lo[:, m:m + 1],
                                 start=False, stop=(k == nmm - 1))
                k += 1
            for m in range(8):
                nc.tensor.matmul(g_ps[:], ctx_lo[:, 128 * m:128 * (m + 1)],
                                 ap_hi[:, m:m + 1],
                                 start=False, stop=(k == nmm - 1))
                k += 1
            g_sb = sp.tile([128, 1], F32, tag="g")
            nc.scalar.activation(g_sb[:], g_ps[:], ACTF.Relu, bias=cbias[:])
            # split g hi/lo bf16
            g_hi = sp.tile([128, 1], BF16, tag="ghi")
            nc.vector.tensor_copy(g_hi[:], g_sb[:])
            g_hif = sp.tile([128, 1], F32, tag="ghif")
            nc.vector.tensor_copy(g_hif[:], g_hi[:])
            g_lof = sp.tile([128, 1], F32, tag="glof")
            nc.vector.tensor_sub(g_lof[:], g_sb[:], g_hif[:])
            g_lo = sp.tile([128, 1], BF16, tag="glo")
            nc.vector.tensor_copy(g_lo[:], g_lof[:])

            # ---------------- GRU partial matvecs (gi; gh ran earlier) ----
            for c in range(24):
                whi = wih_hi[:, 128 * c:128 * (c + 1)]
                nc.tensor.matmul(gih_ps[:, c:c + 1], whi, g_hi[:],
                                 start=True, stop=False)
                nc.tensor.matmul(gih_ps[:, c:c + 1], whi, g_lo[:],
                                 start=False, stop=False)
                nc.tensor.matmul(gih_ps[:, c:c + 1],
                                 wih_lo[:, 128 * c:128 * (c + 1)], g_hi[:],
                                 start=False, stop=True)
            # fold biases/8 in here: the 8-way AllReduce sum restores them
            pack2 = sp.tile([128, 48], F32, tag="pack2")
            nc.vector.tensor_add(pack2[:], gih_ps[:], bias8[:])

            # ---------------- collective 2: AllReduce(add) ----------------
            cc2_in = dp.tile([128, 48], F32, tag="cc2in")
            cc2_out = dp.tile([128, 48], F32, tag="cc2out",
                              addr_space="Shared")
            nc.gpsimd.dma_start(cc2_in[:], pack2[:])
            nc.gpsimd.collective_compute(AR, ADD, replica_groups=RG,
                                         ins=[cc2_in.opt()],
                                         outs=[cc2_out.opt()])
            gsum = sp.tile([128, 48], F32, tag="gsum")
            nc.gpsimd.dma_start(gsum[:], cc2_out[:])

            # gates (PyTorch order r, z, n); r and z share one Sigmoid pass
            rzpre = sp.tile([128, 16], F32, tag="rzpre")
            nc.vector.tensor_add(rzpre[:], gsum[:, 0:16], gsum[:, 24:40])
            rz_sb = sp.tile([128, 16], F32, tag="rz")
            nc.scalar.activation(rz_sb[:], rzpre[:], ACTF.Sigmoid)
            npre = sp.tile([128, 8], F32, tag="npre")
            nc.vector.tensor_mul(npre[:], rz_sb[:, 0:8], gsum[:, 40:48])
            nc.vector.tensor_add(npre[:], npre[:], gsum[:, 16:24])
            n_sb = sp.tile([128, 8], F32, tag="n")
            nc.scalar.activation(n_sb[:], npre[:], ACTF.Tanh)
            # h_new = n + z * (h0 - n)
            hd = sp.tile([128, 8], F32, tag="hd")
            nc.vector.tensor_sub(hd[:], h0cm[:], n_sb[:])
            nc.vector.tensor_mul(hd[:], hd[:], rz_sb[:, 8:16])
            hnew_b = sp.tile([128, 8], BF16, tag="hnewb")
            nc.vector.tensor_add(hnew_b[:], n_sb[:], hd[:])

            # ---------------- output projection ----------------
            logit_sb = sp.tile([128, VT], F32, tag="logit")
            t0c = 0
            while t0c < VT:
                nt = min(8, VT - t0c)
                o_ps = pp.tile([128, nt], F32, tag="ops", bufs=2)
                for t in range(t0c, t0c + nt):
                    for j in range(8):
                        nc.tensor.matmul(
                            o_ps[:, t - t0c:t - t0c + 1],
                            outw[j][:, 128 * t:128 * (t + 1)],
                            hnew_b[:, j:j + 1],
                            start=(j == 0), stop=(j == 7))
                nc.vector.tensor_add(logit_sb[:, t0c:t0c + nt], o_ps[:],
                                     obias[:, t0c:t0c + nt])
                t0c += nt

            nc.gpsimd.dma_start(out_d[:], logit_sb[:])

            if dbg:
                dbg_tiles = {
                    "dbg_a": a_sb, "dbg_exp": exp_sb, "dbg_pack": pack_sb,
                    "dbg_sums": sums, "dbg_cb": cb_sb, "dbg_attnap": attnap,
                    "dbg_g": g_sb, "dbg_pack2": pack2, "dbg_gsum": gsum,
                    "dbg_hnew": hnew_b,
                }
                for name, t in dbg_tiles.items():
                    shp = list(t[:].shape)
                    d = nc.dram_tensor(name, shp, t[:].dtype,
                                       kind="ExternalOutput")
                    nc.sync.dma_start(d[:], t[:])

    nc.compile()
    _CACHE[key] = nc
    return nc


def _col_major(v, ncols):
    # v [n] -> [128, ncols] with [p, c] = v[128 * c + p]
    return np.ascontiguousarray(v.reshape(ncols, 128).T)


def _pack_rows(a, nb):
    # a [nb*128, w] -> [128, nb*w] with [p, w*i + q] = a[128*i + p, q]
    w = a.shape[1]
    return np.ascontiguousarray(
        a.reshape(nb, 128, w).transpose(1, 0, 2).reshape(128, nb * w))


def _hi_lo(a):
    hi = a.astype(NPBF16)
    lo = (a - hi.astype(np.float32)).astype(NPBF16)
    return hi, lo


def _shard(inputs):
    x = np.asarray(inputs["x"]).reshape(-1)
    h0 = np.asarray(inputs["h"], dtype=np.float32).reshape(H)
    enc = np.asarray(inputs["encoder_outputs"], dtype=np.float32)
    emb = np.asarray(inputs["emb"])
    attn_W = np.asarray(inputs["attn_W"], dtype=np.float32)
    attn_b = np.asarray(inputs["attn_b"], dtype=np.float32)
    ctx_W = np.asarray(inputs["ctx_W"], dtype=np.float32)
    ctx_b = np.asarray(inputs["ctx_b"], dtype=np.float32)
    W_ih = np.asarray(inputs["W_ih"], dtype=np.float32)
    W_hh = np.asarray(inputs["W_hh"], dtype=np.float32)
    b_ih = np.asarray(inputs["b_ih"], dtype=np.float32)
    b_hh = np.asarray(inputs["b_hh"], dtype=np.float32)
    out_W = np.asarray(inputs["out_W"], dtype=np.float32)
    out_b = np.asarray(inputs["out_b"], dtype=np.float32)

    xe = np.asarray(emb[int(x[0])], dtype=np.float32)
    catin = np.concatenate([_col_major(xe, 8), _col_major(h0, 8)],
                           axis=1).astype(NPBF16)
    h0cm = _col_major(h0, 8)

    bias8 = np.concatenate([_col_major(b_ih, 24), _col_major(b_hh, 24)],
                           axis=1) / 8.0

    in_maps = []
    for k in range(NCORES):
        lsl = slice(LC * k, LC * (k + 1))
        hsl = slice(HC * k, HC * (k + 1))
        v0, v1 = VC * k, min(VC * (k + 1), V)
        owt = np.zeros((H, VPAD), dtype=NPBF16)
        owt[:, :v1 - v0] = out_W[v0:v1, :].T.astype(NPBF16)
        ob = np.zeros(VPAD, dtype=np.float32)
        ob[:v1 - v0] = out_b[v0:v1]

        enc_hi, enc_lo = _hi_lo(enc[lsl, :])
        ctxT = np.ascontiguousarray(ctx_W[hsl, :].T)       # [2048, 128]
        ctx_hi = ctxT.astype(NPBF16)
        ctx_lo = (ctxT[H:] - ctx_hi[H:].astype(np.float32)).astype(NPBF16)
        wihT = np.ascontiguousarray(W_ih[:, hsl].T)        # [128, 3072]
        wih_hi, wih_lo = _hi_lo(wihT)

        in_maps.append({
            "catin": np.ascontiguousarray(catin),
            "attn_wt": _pack_rows(attn_W[lsl, :].T.astype(NPBF16), 16),
            "attn_b": _col_major(attn_b[lsl], 4),
            "enc_hi": _pack_rows(enc_hi, 4),
            "enc_lo": _pack_rows(enc_lo, 4),
            "ctx_hi": _pack_rows(ctx_hi, 16),
            "ctx_lo": _pack_rows(ctx_lo, 8),
            "ctx_b": ctx_b[hsl].reshape(128, 1).copy(),
            "wih_hi": wih_hi,
            "wih_lo": wih_lo,
            "whh": np.ascontiguousarray(W_hh[:, hsl].T).astype(NPBF16),
            "h0c": h0[hsl].reshape(128, 1).astype(NPBF16),
            "h0cm": h0cm,
            "bias8": np.ascontiguousarray(bias8, dtype=np.float32),
            "out_wt": owt,
            "out_b": _col_major(ob, VT),
        })
    return in_maps


def _gather(results):
    logits = np.empty(NCORES * VC, dtype=np.float32)
    for k in range(NCORES):
        chunk = np.asarray(results[k]["out"]).T.ravel()   # [VT*128]
        logits[VC * k:VC * (k + 1)] = chunk[:VC]
    return logits[:V].reshape(1, V)


def kernel(**inputs):
    nc = _build()
    in_maps = _shard(inputs)
    try:
        res = run_bass_kernel_spmd(nc, in_maps, core_ids=list(range(NCORES)))
    except Exception:
        # A dirty device state from a previous process occasionally fails
        # the first launch (NRT_EXEC_UNIT_UNRECOVERABLE); one retry clears.
        res = run_bass_kernel_spmd(nc, in_maps, core_ids=list(range(NCORES)))
    return _gather(res.results)


def kernel_traced(**inputs):
    """Like kernel() but profiles on HW; returns (output, exec_time_ns)."""
    nc = _build()
    in_maps = _shard(inputs)
    res = run_bass_kernel_spmd(nc, in_maps, core_ids=list(range(NCORES)),
                               trace=True)
    return _gather(res.results), res.exec_time_ns


def kernel_debug(**inputs):
    """Run the debug build; returns per-core dicts of all outputs."""
    nc = _build(dbg=True)
    in_maps = _shard(inputs)
    res = run_bass_kernel_spmd(nc, in_maps, core_ids=list(range(NCORES)))
    return res.results
